# revision 1
# baseline (speedup 1.0000x reference)
"""AriaTextMoELayer on 8 TRN2 NeuronCores — expert-parallel Bass kernel.

Strategy (hardcoded for E=8 experts, TOPK=2, H=1024, I=1024, ISH=2048,
B*S = 2048 tokens, 8 cores):
  - Core e owns expert e: fc1_w[e], fc2_w[e].
  - Shared-expert MLP is tensor-parallel on the intermediate dim:
    core e owns gate_w/up_w[:, 256e:256e+256] and down_w rows [256e:256e+256].
  - hidden_states (transposed to [H, N] on host) and w_router replicated.
  - On device, each core computes router logits for all tokens (fp32, exact),
    derives its expert's per-token top-2 softmax weight w_e with a closed form
    (w_e = [l_e >= m2] * sigmoid(2*l_e - m1 - m2)), runs its expert's SwiGLU
    MLP densely over all tokens (float32r matmuls), scales by w_e (so
    non-routed tokens contribute exactly 0), adds its shared-expert partial,
    and per-half-chunk ReduceScatters over token rows sum the 8 partials.
  - Host reassembles the shards.
"""
import sys

if "/opt/trn_rl_repo" not in sys.path:
    sys.path.insert(0, "/opt/trn_rl_repo")

import numpy as np

from concourse import bacc, bass, mybir, tile
from concourse.masks import make_identity

E = 8
H = 1024
I2 = 2048          # 2*I (fc1 output)
ISH_SH = 256       # shared intermediate shard per core
N = 2048           # tokens
NCORES = 8
TC = 512           # token chunk
NCHUNK = N // TC   # 4
KT = H // 128      # 8 contraction tiles
TT = TC // 128     # 4 token sub-tiles per chunk

F32 = mybir.dt.float32
F32R = mybir.dt.float32r
BF16 = mybir.dt.bfloat16
AX = mybir.AxisListType
OP = mybir.AluOpType
ACTF = mybir.ActivationFunctionType


def build():
    nc = bacc.Bacc(None, target_bir_lowering=False, debug=False)

    xT_d = nc.declare_dram_parameter("xT", [H, N], F32, isOutput=False)
    xb_d = nc.declare_dram_parameter("xb", [H, N], BF16, isOutput=False)
    wr_d = nc.declare_dram_parameter("wr", [H, E], F32, isOutput=False)
    fc1_d = nc.declare_dram_parameter("fc1", [H, I2], BF16, isOutput=False)
    fc2_d = nc.declare_dram_parameter("fc2", [H, H], BF16, isOutput=False)
    gw_d = nc.declare_dram_parameter("gw", [H, ISH_SH], BF16, isOutput=False)
    uw_d = nc.declare_dram_parameter("uw", [H, ISH_SH], BF16, isOutput=False)
    dw_d = nc.declare_dram_parameter("dw", [ISH_SH, H], BF16, isOutput=False)
    esel_d = nc.declare_dram_parameter("esel", [128, TT, E], F32, isOutput=False)
    # per (chunk, half): core r's ReduceScatter shard is [32 tokens, 2, 512]
    out_d = nc.declare_dram_parameter(
        "out", [NCHUNK, 2, 32, 2, 512], BF16, isOutput=True
    )

    with tile.TileContext(nc) as tc:
        with (
            tc.tile_pool(name="wpool", bufs=1) as wpool,
            tc.tile_pool(name="xpool", bufs=2) as xpool,
            tc.tile_pool(name="gpool", bufs=2) as gpool,
            tc.tile_pool(name="shpool", bufs=2) as shpool,
            tc.tile_pool(name="tmppool", bufs=2) as tmppool,
            tc.tile_pool(name="stpool", bufs=3) as stpool,
            tc.tile_pool(name="rpool", bufs=2) as rpool,
            tc.tile_pool(name="psab", bufs=3, space="PSUM") as psab,
            tc.tile_pool(name="psey", bufs=3, space="PSUM") as psey,
            tc.tile_pool(name="psr", bufs=1, space="PSUM") as psr,
            tc.tile_pool(name="dram", bufs=1, space="DRAM") as dram,
        ):
            # contiguous per-(chunk,half) collective buffers (bf16 on the wire;
            # separate tiles so Tile's DRAM dep tracking doesn't serialize
            # chunk c+1's writes behind chunk c's ReduceScatter reads)
            rs_in = [
                dram.tile(
                    [TT, 128, 2, 512], BF16, tag=f"rsin{c}", name=f"rsin{c}"
                )
                for c in range(NCHUNK)
            ]
            rs_out = [
                [
                    dram.tile(
                        [32, 2, 512],
                        BF16,
                        tag=f"rsout{c}_{h}",
                        name=f"rsout{c}_{h}",
                    )
                    for h in range(2)
                ]
                for c in range(NCHUNK)
            ]

            # ---- weights / inputs (DMA emission order = fetch priority) ----
            wr_t = wpool.tile([128, KT, E], F32)
            esel_t = wpool.tile([128, TT, E], F32)
            ident = wpool.tile([E, E], F32)
            nc.sync.dma_start(wr_t[:], wr_d[:].rearrange("(k p) e -> p k e", p=128))
            nc.sync.dma_start(esel_t[:], esel_d[:])
            make_identity(nc, ident[:])

            xT_src = xT_d[:].rearrange("(k p) t -> p k t", p=128)
            xb_src = xb_d[:].rearrange("(k p) t -> p k t", p=128)
            x0_t = xpool.tile([128, KT, TC], F32R, tag="x")
            nc.sync.dma_start(x0_t[:], xT_src[:, :, 0:TC].bitcast(F32R))
            xb0_t = xpool.tile([128, KT, TC], BF16, tag="xb")
            nc.sync.dma_start(xb0_t[:], xb_src[:, :, 0:TC])

            fc1_t = wpool.tile([128, KT, I2], BF16)
            fc1_src = fc1_d[:].rearrange("(k p) o -> p k o", p=128)
            # column pair-groups: group g unlocks proj/gate tile pairs 2g,2g+1
            for g in range(4):
                nc.sync.dma_start(
                    fc1_t[:, :, g * 256 : (g + 1) * 256],
                    fc1_src[:, :, g * 256 : (g + 1) * 256],
                )
                nc.sync.dma_start(
                    fc1_t[:, :, 1024 + g * 256 : 1024 + (g + 1) * 256],
                    fc1_src[:, :, 1024 + g * 256 : 1024 + (g + 1) * 256],
                )

            gw_t = wpool.tile([128, KT, ISH_SH], BF16)
            uw_t = wpool.tile([128, KT, ISH_SH], BF16)
            nc.sync.dma_start(
                gw_t[:], gw_d[:].rearrange("(k p) o -> p k o", p=128)
            )
            nc.sync.dma_start(
                uw_t[:], uw_d[:].rearrange("(k p) o -> p k o", p=128)
            )

            fc2_t = wpool.tile([128, KT, H], BF16)
            fc2_src = fc2_d[:].rearrange("(k p) o -> p k o", p=128)
            for k0 in range(0, KT, 4):
                nc.sync.dma_start(
                    fc2_t[:, k0 : k0 + 4, :],
                    fc2_src[:, k0 : k0 + 4, :],
                )
            dw_t = wpool.tile([128, 2, H], BF16)
            nc.sync.dma_start(
                dw_t[:], dw_d[:].rearrange("(k p) o -> p k o", p=128)
            )

            for c in range(NCHUNK):
                ts, te = c * TC, (c + 1) * TC

                if c == 0:
                    x_t = x0_t
                    xb_t = xb0_t
                else:
                    x_t = xpool.tile([128, KT, TC], F32R, tag="x")
                    nc.sync.dma_start(x_t[:], xT_src[:, :, ts:te].bitcast(F32R))
                    xb_t = xpool.tile([128, KT, TC], BF16, tag="xb")
                    nc.sync.dma_start(xb_t[:], xb_src[:, :, ts:te])
                x_f32 = x_t[:].bitcast(F32)

                # ---- router: expert-major logits, then transpose ----
                lp = psr.tile([E, TC], F32, tag="r")
                for k in range(KT):
                    nc.tensor.matmul(
                        lp[:],
                        wr_t[:, k, :],
                        x_f32[:, k, :],
                        start=(k == 0),
                        stop=(k == KT - 1),
                    )
                l_em = tmppool.tile([E, TC], F32, tag="silu")
                nc.vector.tensor_copy(l_em[:], lp[:])
                logits = rpool.tile([128, TT, E], F32, tag="logits")
                for tt in range(TT):
                    ltp = psr.tile([128, E], F32, tag="rt")
                    nc.tensor.transpose(
                        ltp[:], l_em[:, tt * 128 : (tt + 1) * 128], ident[:]
                    )
                    nc.vector.tensor_copy(logits[:, tt, :], ltp[:])

                # ---- top-2 weight for this core's expert ----
                m8 = rpool.tile([128, TT, 8], F32, tag="m8")
                for tt in range(TT):
                    nc.vector.max(m8[:, tt, :], logits[:, tt, :])
                ltmp = rpool.tile([128, TT, E], F32, tag="ltmp")
                nc.vector.tensor_tensor(ltmp[:], logits[:], esel_t[:], OP.mult)
                le = rpool.tile([128, TT], F32, tag="le")
                nc.vector.tensor_reduce(le[:], ltmp[:], AX.X, OP.add)
                s12 = rpool.tile([128, TT], F32, tag="s12")
                nc.vector.tensor_tensor(
                    s12[:], m8[:, :, 0:1], m8[:, :, 1:2], OP.add
                )
                pre = rpool.tile([128, TT], F32, tag="pre")
                nc.vector.scalar_tensor_tensor(
                    pre[:], le[:], 2.0, s12[:], OP.mult, OP.subtract
                )
                sig = rpool.tile([128, TT], F32, tag="sig")
                nc.scalar.activation(sig[:], pre[:], ACTF.Sigmoid)
                ind = rpool.tile([128, TT], F32, tag="ind")
                nc.vector.tensor_tensor(ind[:], le[:], m8[:, :, 1:2], OP.is_ge)
                w_e = rpool.tile([128, TT], F32, tag="we")
                nc.vector.tensor_tensor(w_e[:], sig[:], ind[:], OP.mult)

                # ---- expert GEMM1 + SwiGLU -> G^T [128, KT(i), TC] f32r ----
                g_t = gpool.tile([128, KT, TC], BF16, tag="g")
                for j in range(KT):  # 8 proj/gate tile pairs
                    pa = psab.tile([128, TC], F32, tag="ab")
                    pb = psab.tile([128, TC], F32, tag="ab")
                    for k in range(KT):
                        nc.tensor.matmul(
                            pa[:],
                            fc1_t[:, k, j * 128 : (j + 1) * 128],
                            xb_t[:, k, :],
                            start=(k == 0),
                            stop=(k == KT - 1),
                        )
                    for k in range(KT):
                        nc.tensor.matmul(
                            pb[:],
                            fc1_t[:, k, 1024 + j * 128 : 1024 + (j + 1) * 128],
                            xb_t[:, k, :],
                            start=(k == 0),
                            stop=(k == KT - 1),
                        )
                    stmp = tmppool.tile([128, TC], F32, tag="silu")
                    nc.scalar.activation(stmp[:], pa[:], ACTF.Silu)
                    nc.vector.tensor_tensor(g_t[:, j, :], stmp[:], pb[:], OP.mult)

                # ---- shared gate/up -> sh^T [128, 2, TC] f32r ----
                sh_t = shpool.tile([128, 2, TC], BF16, tag="sh")
                for o2 in range(2):
                    pg = psab.tile([128, TC], F32, tag="ab")
                    pu = psab.tile([128, TC], F32, tag="ab")
                    for k in range(KT):
                        nc.tensor.matmul(
                            pg[:],
                            gw_t[:, k, o2 * 128 : (o2 + 1) * 128],
                            xb_t[:, k, :],
                            start=(k == 0),
                            stop=(k == KT - 1),
                        )
                    for k in range(KT):
                        nc.tensor.matmul(
                            pu[:],
                            uw_t[:, k, o2 * 128 : (o2 + 1) * 128],
                            xb_t[:, k, :],
                            start=(k == 0),
                            stop=(k == KT - 1),
                        )
                    stmp = tmppool.tile([128, TC], F32, tag="silu")
                    nc.scalar.activation(stmp[:], pg[:], ACTF.Silu)
                    nc.vector.tensor_tensor(sh_t[:, o2, :], stmp[:], pu[:], OP.mult)

                # ---- GEMM2(+down) token-major, scale expert part by w_e ----
                for tt in range(TT):
                    for hh in range(2):
                        hs, he = hh * 512, (hh + 1) * 512
                        pe = psey.tile([128, 512], F32, tag="ey")
                        for i in range(KT):
                            nc.tensor.matmul(
                                pe[:],
                                g_t[:, i, tt * 128 : (tt + 1) * 128],
                                fc2_t[:, i, hs:he],
                                start=(i == 0),
                                stop=(i == KT - 1),
                            )
                        ps = psey.tile([128, 512], F32, tag="ey")
                        for i2 in range(2):
                            nc.tensor.matmul(
                                ps[:],
                                sh_t[:, i2, tt * 128 : (tt + 1) * 128],
                                dw_t[:, i2, hs:he],
                                start=(i2 == 0),
                                stop=(i2 == 1),
                            )
                        stage_f = stpool.tile([128, 512], F32, tag="stf")
                        nc.vector.tensor_scalar(
                            stage_f[:], pe[:], w_e[:, tt : tt + 1], None, OP.mult
                        )
                        stage_b = stpool.tile([128, 512], BF16, tag="stb")
                        nc.vector.tensor_tensor(
                            stage_b[:], stage_f[:], ps[:], OP.add
                        )
                        nc.sync.dma_start(rs_in[c][tt, :, hh, :], stage_b[:])

                    # after each half's stages are out, ReduceScatter that half
                    if tt == 1 or tt == 3:
                        ha = tt // 2
                        nc.gpsimd.collective_compute(
                            "ReduceScatter",
                            OP.add,
                            replica_groups=[list(range(NCORES))],
                            ins=[rs_in[c][2 * ha : 2 * ha + 2].opt()],
                            outs=[rs_out[c][ha].opt()],
                        )
                        nc.sync.dma_start(out_d[c, ha], rs_out[c][ha][:])

    nc.compile()
    return nc


_CACHED = {}


def _prep_in_maps(hidden_states, w_router, fc1_w, fc2_w, gate_w, up_w, down_w):
    import ml_dtypes

    bf16 = ml_dtypes.bfloat16
    xT = np.ascontiguousarray(
        hidden_states.reshape(-1, H).T.astype(np.float32)
    )  # [H, N]
    xb = xT.astype(bf16)
    in_maps = []
    for e in range(NCORES):
        esel = np.zeros((128, TT, E), np.float32)
        esel[:, :, e] = 1.0
        in_maps.append(
            {
                "xT": xT,
                "xb": xb,
                "wr": np.ascontiguousarray(w_router, np.float32),
                "fc1": np.ascontiguousarray(fc1_w[e]).astype(bf16),
                "fc2": np.ascontiguousarray(fc2_w[e]).astype(bf16),
                "gw": np.ascontiguousarray(gate_w[:, e * 256 : (e + 1) * 256]).astype(bf16),
                "uw": np.ascontiguousarray(up_w[:, e * 256 : (e + 1) * 256]).astype(bf16),
                "dw": np.ascontiguousarray(down_w[e * 256 : (e + 1) * 256, :]).astype(bf16),
                "esel": esel,
            }
        )
    return in_maps


def _assemble(results, orig_shape):
    # Core r's shard of (chunk c, half ha) = [32 tokens, 2 h-halves, 512]:
    # tokens [c*512 + (2*ha + r//4)*128 + 32*(r%4) + i], h cols [hh*512 + j].
    full = np.empty((N, H), np.float32)
    for r, res in enumerate(results):
        o = np.asarray(res["out"]).astype(np.float32).reshape(NCHUNK, 2, 32, 2, 512)
        for c in range(NCHUNK):
            for ha in range(2):
                t0 = c * TC + (2 * ha + r // 4) * 128 + 32 * (r % 4)
                blk = o[c, ha]  # [32, 2, 512]
                full[t0 : t0 + 32, 0:512] = blk[:, 0, :]
                full[t0 : t0 + 32, 512:1024] = blk[:, 1, :]
    return full.reshape(orig_shape)


def kernel(hidden_states, w_router, fc1_w, fc2_w, gate_w, up_w, down_w):
    from concourse.bass_utils import run_bass_kernel_spmd

    if "nc" not in _CACHED:
        _CACHED["nc"] = build()
    nc = _CACHED["nc"]
    in_maps = _prep_in_maps(
        hidden_states, w_router, fc1_w, fc2_w, gate_w, up_w, down_w
    )
    res = run_bass_kernel_spmd(nc, in_maps, core_ids=list(range(NCORES)))
    return _assemble(res.results, hidden_states.shape)



# revision 2
# speedup vs baseline: 2.9301x; 2.9301x over previous
"""AriaTextMoELayer on 8 TRN2 NeuronCores — expert-parallel with real
token dispatch.

Sharding strategy (hardcoded for E=8 experts, TOPK=2, H=1024, I=1024,
ISH=2048, B*S = 2048 tokens, 8 cores):
  - The router (logits -> top-2 -> softmax) runs on host as part of
    input sharding: tokens are dispatched (all-to-all style) so core e
    receives exactly the tokens routed to expert e (zero-padded to a
    common capacity `cap`), pre-transposed to [H, cap] bf16.
  - Core e owns expert e's fc1/fc2 and runs the SwiGLU MLP densely over
    its ~cap gathered tokens (vs 2048 dense) — 4x less expert FLOPs.
  - Shared-expert MLP is token-parallel: core e runs the full shared
    SwiGLU for tokens [256e, 256e+256) with replicated gate/up/down.
  - No collectives. Host un-shards: out[tok] = sum_k w_k * yg_ek[tok]
    (router-weighted scatter-add) + shared slice.
"""
import sys

if "/opt/trn_rl_repo" not in sys.path:
    sys.path.insert(0, "/opt/trn_rl_repo")

import numpy as np

from concourse import bacc, bass, mybir, tile

E = 8
TOPK = 2
H = 1024
I = 1024
I2 = 2048          # 2*I (fc1 output: [proj | gate])
ISH = 2048         # shared intermediate
N = 2048           # tokens
SSL = 256          # shared-token slice per core
NCORES = 8
KT = H // 128      # 8 contraction tiles over H
IT = I // 128      # 8 contraction tiles over I
ST = ISH // 128    # 16 tiles over shared intermediate

F32 = mybir.dt.float32
BF16 = mybir.dt.bfloat16
OP = mybir.AluOpType
ACTF = mybir.ActivationFunctionType


def _chunks(n, c=512):
    out = []
    s = 0
    while s < n:
        out.append((s, min(s + c, n)))
        s += c
    return out


def build(cap):
    nc = bacc.Bacc(None, target_bir_lowering=False, debug=False)

    xg_d = nc.declare_dram_parameter("xg", [H, cap], BF16, isOutput=False)
    xs_d = nc.declare_dram_parameter("xs", [H, SSL], BF16, isOutput=False)
    fc1_d = nc.declare_dram_parameter("fc1", [H, I2], BF16, isOutput=False)
    fc2_d = nc.declare_dram_parameter("fc2", [I, H], BF16, isOutput=False)
    gw_d = nc.declare_dram_parameter("gw", [H, ISH], BF16, isOutput=False)
    uw_d = nc.declare_dram_parameter("uw", [H, ISH], BF16, isOutput=False)
    dw_d = nc.declare_dram_parameter("dw", [ISH, H], BF16, isOutput=False)
    yg_d = nc.declare_dram_parameter("yg", [cap, H], BF16, isOutput=True)
    ys_d = nc.declare_dram_parameter("ys", [SSL, H], BF16, isOutput=True)

    nt = cap // 128  # token tiles for expert GEMM2 (cap is 128-aligned)

    with tile.TileContext(nc) as tc:
        with (
            tc.tile_pool(name="wpool", bufs=1) as wpool,
            tc.tile_pool(name="xpool", bufs=1) as xpool,
            tc.tile_pool(name="gpool", bufs=1) as gpool,
            tc.tile_pool(name="tmppool", bufs=3) as tmppool,
            tc.tile_pool(name="stpool", bufs=4) as stpool,
            tc.tile_pool(name="psab", bufs=4, space="PSUM") as psab,
            tc.tile_pool(name="psey", bufs=4, space="PSUM") as psey,
        ):
            # ---- input/weight DMAs (emission order = fetch priority) ----
            xg_t = xpool.tile([128, KT, cap], BF16)
            nc.sync.dma_start(
                xg_t[:], xg_d[:].rearrange("(k p) t -> p k t", p=128)
            )
            fc1_t = wpool.tile([128, KT, I2], BF16)
            fc1_src = fc1_d[:].rearrange("(k p) o -> p k o", p=128)
            # proj/gate column pair-groups: group g unlocks SwiGLU pairs
            for g in range(4):
                nc.sync.dma_start(
                    fc1_t[:, :, g * 256 : (g + 1) * 256],
                    fc1_src[:, :, g * 256 : (g + 1) * 256],
                )
                nc.sync.dma_start(
                    fc1_t[:, :, 1024 + g * 256 : 1024 + (g + 1) * 256],
                    fc1_src[:, :, 1024 + g * 256 : 1024 + (g + 1) * 256],
                )
            xs_t = xpool.tile([128, KT, SSL], BF16)
            nc.sync.dma_start(
                xs_t[:], xs_d[:].rearrange("(k p) t -> p k t", p=128)
            )
            gw_t = wpool.tile([128, KT, ISH], BF16)
            uw_t = wpool.tile([128, KT, ISH], BF16)
            gw_src = gw_d[:].rearrange("(k p) o -> p k o", p=128)
            uw_src = uw_d[:].rearrange("(k p) o -> p k o", p=128)
            for g in range(4):
                sl = slice(g * 512, (g + 1) * 512)
                nc.sync.dma_start(gw_t[:, :, sl], gw_src[:, :, sl])
                nc.sync.dma_start(uw_t[:, :, sl], uw_src[:, :, sl])
            fc2_t = wpool.tile([128, IT, H], BF16)
            fc2_src = fc2_d[:].rearrange("(k p) o -> p k o", p=128)
            for k0 in range(0, IT, 4):
                nc.sync.dma_start(
                    fc2_t[:, k0 : k0 + 4, :], fc2_src[:, k0 : k0 + 4, :]
                )
            dw_t = wpool.tile([128, ST, H], BF16)
            dw_src = dw_d[:].rearrange("(k p) o -> p k o", p=128)
            for k0 in range(0, ST, 8):
                nc.sync.dma_start(
                    dw_t[:, k0 : k0 + 8, :], dw_src[:, k0 : k0 + 8, :]
                )

            # ---- expert GEMM1 + SwiGLU -> g_t [128, IT(i), cap] bf16 ----
            g_t = gpool.tile([128, IT, cap], BF16)
            for j in range(IT):  # 8 proj/gate 128-col pairs
                for ts, te in _chunks(cap):
                    csz = te - ts
                    pa = psab.tile([128, csz], F32, tag="ab")
                    for k in range(KT):
                        nc.tensor.matmul(
                            pa[:],
                            fc1_t[:, k, j * 128 : (j + 1) * 128],
                            xg_t[:, k, ts:te],
                            start=(k == 0),
                            stop=(k == KT - 1),
                        )
                    pb = psab.tile([128, csz], F32, tag="ab")
                    for k in range(KT):
                        nc.tensor.matmul(
                            pb[:],
                            fc1_t[:, k, 1024 + j * 128 : 1024 + (j + 1) * 128],
                            xg_t[:, k, ts:te],
                            start=(k == 0),
                            stop=(k == KT - 1),
                        )
                    stmp = tmppool.tile([128, csz], F32, tag="silu")
                    nc.scalar.activation(stmp[:], pa[:], ACTF.Silu)
                    nc.vector.tensor_tensor(
                        g_t[:, j, ts:te], stmp[:], pb[:], OP.mult
                    )

            # ---- shared GEMM1 + SwiGLU -> sh_t [128, ST(i), SSL] bf16 ----
            # (emitted between expert G1 and G2 so PE stays busy while the
            # last g_t tiles drain through ACT/DVE)
            sh_t = gpool.tile([128, ST, SSL], BF16)
            for o in range(ST):  # 16 gate/up 128-col pairs
                pg = psab.tile([128, SSL], F32, tag="ab")
                for k in range(KT):
                    nc.tensor.matmul(
                        pg[:],
                        gw_t[:, k, o * 128 : (o + 1) * 128],
                        xs_t[:, k, :],
                        start=(k == 0),
                        stop=(k == KT - 1),
                    )
                pu = psab.tile([128, SSL], F32, tag="ab")
                for k in range(KT):
                    nc.tensor.matmul(
                        pu[:],
                        uw_t[:, k, o * 128 : (o + 1) * 128],
                        xs_t[:, k, :],
                        start=(k == 0),
                        stop=(k == KT - 1),
                    )
                stmp = tmppool.tile([128, SSL], F32, tag="silu")
                nc.scalar.activation(stmp[:], pg[:], ACTF.Silu)
                nc.vector.tensor_tensor(
                    sh_t[:, o, :], stmp[:], pu[:], OP.mult
                )

            # ---- expert GEMM2: yg[t, :] = g_t[:, :, t].T @ fc2 ----
            for t in range(nt):
                t0 = t * 128
                pe0 = psey.tile([128, 512], F32, tag="ey")
                pe1 = psey.tile([128, 512], F32, tag="ey")
                for i in range(IT):
                    nc.tensor.matmul(
                        pe0[:],
                        g_t[:, i, t0 : t0 + 128],
                        fc2_t[:, i, 0:512],
                        start=(i == 0),
                        stop=(i == IT - 1),
                    )
                    nc.tensor.matmul(
                        pe1[:],
                        g_t[:, i, t0 : t0 + 128],
                        fc2_t[:, i, 512:1024],
                        start=(i == 0),
                        stop=(i == IT - 1),
                    )
                st0 = stpool.tile([128, 512], BF16, tag="st")
                nc.vector.tensor_copy(st0[:], pe0[:])
                nc.sync.dma_start(yg_d[t0 : t0 + 128, 0:512], st0[:])
                st1 = stpool.tile([128, 512], BF16, tag="st")
                nc.vector.tensor_copy(st1[:], pe1[:])
                nc.sync.dma_start(yg_d[t0 : t0 + 128, 512:1024], st1[:])

            # ---- shared down: ys[t, :] = sh_t[:, :, t].T @ dw ----
            for t in range(SSL // 128):
                t0 = t * 128
                pd0 = psey.tile([128, 512], F32, tag="ey")
                pd1 = psey.tile([128, 512], F32, tag="ey")
                for i in range(ST):
                    nc.tensor.matmul(
                        pd0[:],
                        sh_t[:, i, t0 : t0 + 128],
                        dw_t[:, i, 0:512],
                        start=(i == 0),
                        stop=(i == ST - 1),
                    )
                    nc.tensor.matmul(
                        pd1[:],
                        sh_t[:, i, t0 : t0 + 128],
                        dw_t[:, i, 512:1024],
                        start=(i == 0),
                        stop=(i == ST - 1),
                    )
                st0 = stpool.tile([128, 512], BF16, tag="st")
                nc.vector.tensor_copy(st0[:], pd0[:])
                nc.sync.dma_start(ys_d[t0 : t0 + 128, 0:512], st0[:])
                st1 = stpool.tile([128, 512], BF16, tag="st")
                nc.vector.tensor_copy(st1[:], pd1[:])
                nc.sync.dma_start(ys_d[t0 : t0 + 128, 512:1024], st1[:])

    nc.compile()
    return nc


_CACHED = {}


def _route(x, w_router):
    """Host router: top-2 indices (ties -> lower index, like lax.top_k)
    and softmax weights over the top-2 logits."""
    logits = x.astype(np.float32) @ w_router.astype(np.float32)  # [N, E]
    top2 = np.argsort(-logits, axis=1, kind="stable")[:, :TOPK]  # [N, 2]
    l2 = np.take_along_axis(logits, top2, axis=1)
    m = l2.max(axis=1, keepdims=True)
    ex = np.exp(l2 - m)
    w = ex / ex.sum(axis=1, keepdims=True)
    return top2, w


def _prep(hidden_states, w_router, fc1_w, fc2_w, gate_w, up_w, down_w):
    import ml_dtypes

    bf16 = ml_dtypes.bfloat16
    x = np.ascontiguousarray(hidden_states.reshape(-1, H), dtype=np.float32)
    top2, w = _route(x, w_router)

    tok_lists = []
    wt_lists = []
    for e in range(NCORES):
        sel = np.where((top2[:, 0] == e) | (top2[:, 1] == e))[0]
        tok_lists.append(sel)
        wt_lists.append(np.where(top2[sel, 0] == e, w[sel, 0], w[sel, 1]))
    max_cnt = max(len(s) for s in tok_lists)
    cap = max(128, -(-max_cnt // 128) * 128)

    xb = x.astype(bf16)
    in_maps = []
    for e in range(NCORES):
        sel = tok_lists[e]
        xgT = np.zeros((H, cap), dtype=bf16)
        xgT[:, : len(sel)] = xb[sel].T
        xsT = np.ascontiguousarray(xb[e * SSL : (e + 1) * SSL].T)
        in_maps.append(
            {
                "xg": xgT,
                "xs": xsT,
                "fc1": np.ascontiguousarray(fc1_w[e]).astype(bf16),
                "fc2": np.ascontiguousarray(fc2_w[e]).astype(bf16),
                "gw": np.ascontiguousarray(gate_w).astype(bf16),
                "uw": np.ascontiguousarray(up_w).astype(bf16),
                "dw": np.ascontiguousarray(down_w).astype(bf16),
            }
        )
    return cap, in_maps, tok_lists, wt_lists


def _assemble(results, tok_lists, wt_lists, orig_shape):
    out = np.zeros((N, H), dtype=np.float32)
    for e, res in enumerate(results):
        out[e * SSL : (e + 1) * SSL] = np.asarray(res["ys"]).astype(np.float32)
    for e, res in enumerate(results):
        sel = tok_lists[e]
        if len(sel) == 0:
            continue
        yg = np.asarray(res["yg"])[: len(sel)].astype(np.float32)
        out[sel] += wt_lists[e][:, None] * yg
    return out.reshape(orig_shape)


def kernel(hidden_states, w_router, fc1_w, fc2_w, gate_w, up_w, down_w):
    from concourse.bass_utils import run_bass_kernel_spmd

    cap, in_maps, tok_lists, wt_lists = _prep(
        hidden_states, w_router, fc1_w, fc2_w, gate_w, up_w, down_w
    )
    if cap not in _CACHED:
        _CACHED[cap] = build(cap)
    nc = _CACHED[cap]
    res = run_bass_kernel_spmd(nc, in_maps, core_ids=list(range(NCORES)))
    return _assemble(res.results, tok_lists, wt_lists, hidden_states.shape)


# revision 3
# speedup vs baseline: 3.0491x; 1.0406x over previous
"""AriaTextMoELayer on 8 TRN2 NeuronCores — expert-parallel with real
token dispatch.

Sharding strategy (hardcoded for E=8 experts, TOPK=2, H=1024, I=1024,
ISH=2048, B*S = 2048 tokens, 8 cores):
  - The router (logits -> top-2 -> softmax) runs on host as part of
    input sharding: tokens are dispatched (all-to-all style) so core e
    receives exactly the tokens routed to expert e (zero-padded to a
    common capacity `cap`), pre-transposed into device tile layout.
  - Core e owns expert e's fc1/fc2 and runs the SwiGLU MLP densely over
    its ~cap gathered tokens (vs 2048 dense) — 4x less expert FLOPs.
  - Shared-expert MLP is token-parallel: core e runs the full shared
    SwiGLU for tokens [256e, 256e+256) with replicated gate/up/down.
    It is computed FIRST on device (needs only 1MB of DMA to start)
    while the expert weights stream in behind it.
  - No collectives. Host un-shards: out[tok] = sum_k w_k * yg_ek[tok]
    (router-weighted scatter-add) + shared slice.

All host->device tensors are pre-shuffled on host into the exact SBUF
tile layout ([128 partitions, ktile, cols], proj/gate and gate/up pairs
interleaved per 128-col group) so every DMA is a contiguous full-BW
block copy and each 0.5MB chunk unlocks one SwiGLU pair of compute.
"""
import sys

if "/opt/trn_rl_repo" not in sys.path:
    sys.path.insert(0, "/opt/trn_rl_repo")

import numpy as np

from concourse import bacc, bass, mybir, tile

E = 8
TOPK = 2
H = 1024
I = 1024
I2 = 2048          # 2*I (fc1 output: [proj | gate])
ISH = 2048         # shared intermediate
N = 2048           # tokens
SSL = 256          # shared-token slice per core
NCORES = 8
KT = H // 128      # 8 contraction tiles over H
IT = I // 128      # 8 contraction tiles over I
ST = ISH // 128    # 16 tiles over shared intermediate

F32 = mybir.dt.float32
BF16 = mybir.dt.bfloat16
OP = mybir.AluOpType
ACTF = mybir.ActivationFunctionType


def _chunks(n, c=512):
    out = []
    s = 0
    while s < n:
        out.append((s, min(s + c, n)))
        s += c
    return out


def build(cap):
    nc = bacc.Bacc(None, target_bir_lowering=False, debug=False)

    xg_d = nc.declare_dram_parameter("xg", [128, KT, cap], BF16, isOutput=False)
    xs_d = nc.declare_dram_parameter("xs", [128, KT, SSL], BF16, isOutput=False)
    fc1_d = nc.declare_dram_parameter(
        "fc1", [IT, 128, KT, 256], BF16, isOutput=False
    )
    fc2_d = nc.declare_dram_parameter("fc2", [128, IT, H], BF16, isOutput=False)
    gwu_d = nc.declare_dram_parameter(
        "gwu", [ST, 128, KT, 256], BF16, isOutput=False
    )
    dw_d = nc.declare_dram_parameter("dw", [128, ST, H], BF16, isOutput=False)
    yg_d = nc.declare_dram_parameter("yg", [cap, H], BF16, isOutput=True)
    ys_d = nc.declare_dram_parameter("ys", [SSL, H], BF16, isOutput=True)

    nt = cap // 128  # token tiles for expert GEMM2 (cap is 128-aligned)

    with tile.TileContext(nc) as tc:
        with (
            tc.tile_pool(name="wpool", bufs=1) as wpool,
            tc.tile_pool(name="xpool", bufs=1) as xpool,
            tc.tile_pool(name="gpool", bufs=1) as gpool,
            tc.tile_pool(name="tmppool", bufs=3) as tmppool,
            tc.tile_pool(name="stpool", bufs=4) as stpool,
            tc.tile_pool(name="psab", bufs=4, space="PSUM") as psab,
            tc.tile_pool(name="psey", bufs=4, space="PSUM") as psey,
        ):
            # ---- DMAs (emission order = fetch priority) ----
            # shared inputs first: the shared phase starts after ~1MB.
            xs_t = xpool.tile([128, KT, SSL], BF16)
            nc.sync.dma_start(xs_t[:], xs_d[:])
            gwu_t = wpool.tile([128, ST, KT, 256], BF16)
            for o in range(ST):
                nc.sync.dma_start(gwu_t[:, o], gwu_d[o])
            xg_t = xpool.tile([128, KT, cap], BF16)
            nc.sync.dma_start(xg_t[:], xg_d[:])
            fc1_t = wpool.tile([128, IT, KT, 256], BF16)
            for j in range(IT):
                nc.sync.dma_start(fc1_t[:, j], fc1_d[j])
            fc2_t = wpool.tile([128, IT, H], BF16)
            for k0 in range(0, IT, 4):
                nc.sync.dma_start(
                    fc2_t[:, k0 : k0 + 4, :], fc2_d[:, k0 : k0 + 4, :]
                )
            dw_t = wpool.tile([128, ST, H], BF16)
            for k0 in range(0, ST, 8):
                nc.sync.dma_start(
                    dw_t[:, k0 : k0 + 8, :], dw_d[:, k0 : k0 + 8, :]
                )

            # ---- shared GEMM1 + SwiGLU -> sh_t [128, ST(i), SSL] bf16 ----
            sh_t = gpool.tile([128, ST, SSL], BF16)
            for o in range(ST):  # 16 gate/up 128-col pairs
                pg = psab.tile([128, SSL], F32, tag="ab")
                for k in range(KT):
                    nc.tensor.matmul(
                        pg[:],
                        gwu_t[:, o, k, 0:128],
                        xs_t[:, k, :],
                        start=(k == 0),
                        stop=(k == KT - 1),
                    )
                pu = psab.tile([128, SSL], F32, tag="ab")
                for k in range(KT):
                    nc.tensor.matmul(
                        pu[:],
                        gwu_t[:, o, k, 128:256],
                        xs_t[:, k, :],
                        start=(k == 0),
                        stop=(k == KT - 1),
                    )
                stmp = tmppool.tile([128, SSL], F32, tag="silu")
                nc.scalar.activation(stmp[:], pg[:], ACTF.Silu)
                nc.vector.tensor_tensor(
                    sh_t[:, o, :], stmp[:], pu[:], OP.mult
                )

            # ---- expert GEMM1 + SwiGLU -> g_t [128, IT(i), cap] bf16 ----
            g_t = gpool.tile([128, IT, cap], BF16)
            for j in range(IT):  # 8 proj/gate 128-col pairs
                for ts, te in _chunks(cap):
                    csz = te - ts
                    pa = psab.tile([128, csz], F32, tag="ab")
                    for k in range(KT):
                        nc.tensor.matmul(
                            pa[:],
                            fc1_t[:, j, k, 0:128],
                            xg_t[:, k, ts:te],
                            start=(k == 0),
                            stop=(k == KT - 1),
                        )
                    pb = psab.tile([128, csz], F32, tag="ab")
                    for k in range(KT):
                        nc.tensor.matmul(
                            pb[:],
                            fc1_t[:, j, k, 128:256],
                            xg_t[:, k, ts:te],
                            start=(k == 0),
                            stop=(k == KT - 1),
                        )
                    stmp = tmppool.tile([128, csz], F32, tag="silu")
                    nc.scalar.activation(stmp[:], pa[:], ACTF.Silu)
                    nc.vector.tensor_tensor(
                        g_t[:, j, ts:te], stmp[:], pb[:], OP.mult
                    )

            # ---- expert GEMM2: yg[t, :] = g_t[:, :, t].T @ fc2 ----
            for t in range(nt):
                t0 = t * 128
                pe0 = psey.tile([128, 512], F32, tag="ey")
                pe1 = psey.tile([128, 512], F32, tag="ey")
                for i in range(IT):
                    nc.tensor.matmul(
                        pe0[:],
                        g_t[:, i, t0 : t0 + 128],
                        fc2_t[:, i, 0:512],
                        start=(i == 0),
                        stop=(i == IT - 1),
                    )
                    nc.tensor.matmul(
                        pe1[:],
                        g_t[:, i, t0 : t0 + 128],
                        fc2_t[:, i, 512:1024],
                        start=(i == 0),
                        stop=(i == IT - 1),
                    )
                st0 = stpool.tile([128, 512], BF16, tag="st")
                nc.vector.tensor_copy(st0[:], pe0[:])
                nc.sync.dma_start(yg_d[t0 : t0 + 128, 0:512], st0[:])
                st1 = stpool.tile([128, 512], BF16, tag="st")
                nc.vector.tensor_copy(st1[:], pe1[:])
                nc.sync.dma_start(yg_d[t0 : t0 + 128, 512:1024], st1[:])

            # ---- shared down: ys[t, :] = sh_t[:, :, t].T @ dw ----
            for t in range(SSL // 128):
                t0 = t * 128
                pd0 = psey.tile([128, 512], F32, tag="ey")
                pd1 = psey.tile([128, 512], F32, tag="ey")
                for i in range(ST):
                    nc.tensor.matmul(
                        pd0[:],
                        sh_t[:, i, t0 : t0 + 128],
                        dw_t[:, i, 0:512],
                        start=(i == 0),
                        stop=(i == ST - 1),
                    )
                    nc.tensor.matmul(
                        pd1[:],
                        sh_t[:, i, t0 : t0 + 128],
                        dw_t[:, i, 512:1024],
                        start=(i == 0),
                        stop=(i == ST - 1),
                    )
                st0 = stpool.tile([128, 512], BF16, tag="st")
                nc.vector.tensor_copy(st0[:], pd0[:])
                nc.sync.dma_start(ys_d[t0 : t0 + 128, 0:512], st0[:])
                st1 = stpool.tile([128, 512], BF16, tag="st")
                nc.vector.tensor_copy(st1[:], pd1[:])
                nc.sync.dma_start(ys_d[t0 : t0 + 128, 512:1024], st1[:])

    nc.compile()
    return nc


_CACHED = {}


def _route(x, w_router):
    """Host router: top-2 indices (ties -> lower index, like lax.top_k)
    and softmax weights over the top-2 logits."""
    logits = x.astype(np.float32) @ w_router.astype(np.float32)  # [N, E]
    top2 = np.argsort(-logits, axis=1, kind="stable")[:, :TOPK]  # [N, 2]
    l2 = np.take_along_axis(logits, top2, axis=1)
    m = l2.max(axis=1, keepdims=True)
    ex = np.exp(l2 - m)
    w = ex / ex.sum(axis=1, keepdims=True)
    return top2, w


def _km(a):
    """[H_like, C] -> [128, H_like//128, C] k-major contiguous."""
    kt = a.shape[0] // 128
    return np.ascontiguousarray(a.reshape(kt, 128, a.shape[1]).transpose(1, 0, 2))


def _prep(hidden_states, w_router, fc1_w, fc2_w, gate_w, up_w, down_w):
    import ml_dtypes

    bf16 = ml_dtypes.bfloat16
    x = np.ascontiguousarray(hidden_states.reshape(-1, H), dtype=np.float32)
    top2, w = _route(x, w_router)

    tok_lists = []
    wt_lists = []
    for e in range(NCORES):
        sel = np.where((top2[:, 0] == e) | (top2[:, 1] == e))[0]
        tok_lists.append(sel)
        wt_lists.append(np.where(top2[sel, 0] == e, w[sel, 0], w[sel, 1]))
    max_cnt = max(len(s) for s in tok_lists)
    cap = max(128, -(-max_cnt // 128) * 128)

    xb = x.astype(bf16)
    # gate/up 128-col pairs, shared across cores: [ST, 128, KT, 256]
    gu = np.empty((ST, 128, KT, 256), dtype=bf16)
    gwb = gate_w.astype(bf16)
    uwb = up_w.astype(bf16)
    for o in range(ST):
        gu[o, :, :, 0:128] = _km(gwb[:, o * 128 : (o + 1) * 128])
        gu[o, :, :, 128:256] = _km(uwb[:, o * 128 : (o + 1) * 128])
    dwk = _km(down_w.astype(bf16))  # [128, ST, H]

    in_maps = []
    for e in range(NCORES):
        sel = tok_lists[e]
        xgT = np.zeros((H, cap), dtype=bf16)
        xgT[:, : len(sel)] = xb[sel].T
        f1b = fc1_w[e].astype(bf16)
        f1 = np.empty((IT, 128, KT, 256), dtype=bf16)
        for j in range(IT):
            f1[j, :, :, 0:128] = _km(f1b[:, j * 128 : (j + 1) * 128])
            f1[j, :, :, 128:256] = _km(f1b[:, 1024 + j * 128 : 1024 + (j + 1) * 128])
        in_maps.append(
            {
                "xg": _km(xgT),
                "xs": _km(np.ascontiguousarray(xb[e * SSL : (e + 1) * SSL].T)),
                "fc1": f1,
                "fc2": _km(fc2_w[e].astype(bf16)),
                "gwu": gu,
                "dw": dwk,
            }
        )
    return cap, in_maps, tok_lists, wt_lists


def _assemble(results, tok_lists, wt_lists, orig_shape):
    out = np.zeros((N, H), dtype=np.float32)
    for e, res in enumerate(results):
        out[e * SSL : (e + 1) * SSL] = np.asarray(res["ys"]).astype(np.float32)
    for e, res in enumerate(results):
        sel = tok_lists[e]
        if len(sel) == 0:
            continue
        yg = np.asarray(res["yg"])[: len(sel)].astype(np.float32)
        out[sel] += wt_lists[e][:, None] * yg
    return out.reshape(orig_shape)


def kernel(hidden_states, w_router, fc1_w, fc2_w, gate_w, up_w, down_w):
    from concourse.bass_utils import run_bass_kernel_spmd

    cap, in_maps, tok_lists, wt_lists = _prep(
        hidden_states, w_router, fc1_w, fc2_w, gate_w, up_w, down_w
    )
    if cap not in _CACHED:
        _CACHED[cap] = build(cap)
    nc = _CACHED[cap]
    res = run_bass_kernel_spmd(nc, in_maps, core_ids=list(range(NCORES)))
    return _assemble(res.results, tok_lists, wt_lists, hidden_states.shape)


# revision 8
# speedup vs baseline: 3.1331x; 1.0276x over previous
"""AriaTextMoELayer on 8 TRN2 NeuronCores — expert-parallel with real
token dispatch.

Sharding strategy (hardcoded for E=8 experts, TOPK=2, H=1024, I=1024,
ISH=2048, B*S = 2048 tokens, 8 cores):
  - The router (logits -> top-2 -> softmax) runs on host as part of
    input sharding: tokens are dispatched (all-to-all style) so core e
    receives exactly the tokens routed to expert e (zero-padded to a
    common capacity `cap`), pre-transposed into device tile layout.
  - Core e owns expert e's fc1/fc2 and runs the SwiGLU MLP densely over
    its ~cap gathered tokens (vs 2048 dense) — 4x less expert FLOPs.
  - Shared-expert MLP is token-parallel: core e runs the full shared
    SwiGLU for tokens [256e, 256e+256) with replicated gate/up/down.
    It is computed FIRST on device (needs only 1MB of DMA to start)
    while the expert weights stream in behind it.
  - No collectives. Host un-shards: out[tok] = sum_k w_k * yg_ek[tok]
    (router-weighted scatter-add) + shared slice.

All host->device tensors are pre-shuffled on host into the exact SBUF
tile layout ([128 partitions, ktile, cols], proj/gate and gate/up pairs
interleaved per 128-col group) so every DMA is a contiguous full-BW
block copy and each 0.5MB chunk unlocks one SwiGLU pair of compute.
"""
import sys

if "/opt/trn_rl_repo" not in sys.path:
    sys.path.insert(0, "/opt/trn_rl_repo")

import numpy as np

from concourse import bacc, bass, mybir, tile

E = 8
TOPK = 2
H = 1024
I = 1024
I2 = 2048          # 2*I (fc1 output: [proj | gate])
ISH = 2048         # shared intermediate
N = 2048           # tokens
SSL = 256          # shared-token slice per core
NCORES = 8
KT = H // 128      # 8 contraction tiles over H
IT = I // 128      # 8 contraction tiles over I
ST = ISH // 128    # 16 tiles over shared intermediate

F32 = mybir.dt.float32
BF16 = mybir.dt.bfloat16
OP = mybir.AluOpType
ACTF = mybir.ActivationFunctionType


def _chunks(n, c=512):
    out = []
    s = 0
    while s < n:
        out.append((s, min(s + c, n)))
        s += c
    return out


def build(cap):
    nc = bacc.Bacc(None, target_bir_lowering=False, debug=False)

    xg_d = nc.declare_dram_parameter("xg", [128, KT, cap], BF16, isOutput=False)
    xs_d = nc.declare_dram_parameter("xs", [128, KT, SSL], BF16, isOutput=False)
    fc1_d = nc.declare_dram_parameter(
        "fc1", [IT, 128, KT, 256], BF16, isOutput=False
    )
    fc2_d = nc.declare_dram_parameter("fc2", [128, IT, H], BF16, isOutput=False)
    gwu_d = nc.declare_dram_parameter(
        "gwu", [ST, 128, KT, 256], BF16, isOutput=False
    )
    dw_d = nc.declare_dram_parameter("dw", [128, ST, H], BF16, isOutput=False)
    yg_d = nc.declare_dram_parameter("yg", [cap, H], BF16, isOutput=True)
    ys_d = nc.declare_dram_parameter("ys", [SSL, H], BF16, isOutput=True)

    nt = -(-cap // 128)  # token tiles for expert GEMM2

    with tile.TileContext(nc) as tc:
        with (
            tc.tile_pool(name="wpool", bufs=1) as wpool,
            tc.tile_pool(name="xpool", bufs=1) as xpool,
            tc.tile_pool(name="gpool", bufs=1) as gpool,
            tc.tile_pool(name="tmppool", bufs=3) as tmppool,
            tc.tile_pool(name="stpool", bufs=4) as stpool,
            tc.tile_pool(name="psab", bufs=4, space="PSUM") as psab,
            tc.tile_pool(name="psey", bufs=4, space="PSUM") as psey,
        ):
            # ---- DMAs (emission order = fetch priority, single HWDGE
            # queue: HBM BW is shared, so strict priority order beats
            # parallel queues). First chunks k-sliced so the first
            # matmul starts ASAP. ----
            xs_t = xpool.tile([128, KT, SSL], BF16)
            nc.sync.dma_start(xs_t[:, 0:4, :], xs_d[:, 0:4, :])
            gwu_t = wpool.tile([128, ST, KT, 256], BF16)
            nc.sync.dma_start(gwu_t[:, 0, 0:4], gwu_d[0, :, 0:4])
            nc.sync.dma_start(xs_t[:, 4:8, :], xs_d[:, 4:8, :])
            nc.sync.dma_start(gwu_t[:, 0, 4:8], gwu_d[0, :, 4:8])
            for o in range(1, ST):
                nc.sync.dma_start(gwu_t[:, o], gwu_d[o])
            xg_t = xpool.tile([128, KT, cap], BF16)
            nc.sync.dma_start(xg_t[:], xg_d[:])
            fc1_t = wpool.tile([128, IT, KT, 256], BF16)
            for j in range(IT):
                nc.sync.dma_start(fc1_t[:, j], fc1_d[j])
            fc2_t = wpool.tile([128, IT, H], BF16)
            for k0 in range(0, IT, 4):
                nc.sync.dma_start(
                    fc2_t[:, k0 : k0 + 4, :], fc2_d[:, k0 : k0 + 4, :]
                )
            dw_t = wpool.tile([128, ST, H], BF16)
            for k0 in range(0, ST, 8):
                nc.sync.dma_start(
                    dw_t[:, k0 : k0 + 8, :], dw_d[:, k0 : k0 + 8, :]
                )

            # ---- shared GEMM1 + SwiGLU -> sh_t [128, ST(i), SSL] bf16 ----
            sh_t = gpool.tile([128, ST, SSL], BF16)
            for o in range(ST):  # 16 gate/up 128-col pairs
                pg = psab.tile([128, SSL], F32, tag="ab")
                for k in range(KT):
                    nc.tensor.matmul(
                        pg[:],
                        gwu_t[:, o, k, 0:128],
                        xs_t[:, k, :],
                        start=(k == 0),
                        stop=(k == KT - 1),
                    )
                pu = psab.tile([128, SSL], F32, tag="ab")
                for k in range(KT):
                    nc.tensor.matmul(
                        pu[:],
                        gwu_t[:, o, k, 128:256],
                        xs_t[:, k, :],
                        start=(k == 0),
                        stop=(k == KT - 1),
                    )
                stmp = tmppool.tile([128, SSL], F32, tag="silu")
                nc.scalar.activation(stmp[:], pg[:], ACTF.Silu)
                nc.vector.tensor_tensor(
                    sh_t[:, o, :], stmp[:], pu[:], OP.mult
                )

            # ---- expert GEMM1 + SwiGLU -> g_t [128, IT(i), cap] bf16 ----
            g_t = gpool.tile([128, IT, cap], BF16)
            for j in range(IT):  # 8 proj/gate 128-col pairs
                for ts, te in _chunks(cap):
                    csz = te - ts
                    pa = psab.tile([128, csz], F32, tag="ab")
                    for k in range(KT):
                        nc.tensor.matmul(
                            pa[:],
                            fc1_t[:, j, k, 0:128],
                            xg_t[:, k, ts:te],
                            start=(k == 0),
                            stop=(k == KT - 1),
                        )
                    pb = psab.tile([128, csz], F32, tag="ab")
                    for k in range(KT):
                        nc.tensor.matmul(
                            pb[:],
                            fc1_t[:, j, k, 128:256],
                            xg_t[:, k, ts:te],
                            start=(k == 0),
                            stop=(k == KT - 1),
                        )
                    stmp = tmppool.tile([128, csz], F32, tag="silu")
                    nc.scalar.activation(stmp[:], pa[:], ACTF.Silu)
                    nc.vector.tensor_tensor(
                        g_t[:, j, ts:te], stmp[:], pb[:], OP.mult
                    )

            # ---- expert GEMM2: yg[t, :] = g_t[:, :, t].T @ fc2 ----
            for t in range(nt):
                t0 = t * 128
                rows = min(128, cap - t0)
                pe0 = psey.tile([rows, 512], F32, tag="ey")
                pe1 = psey.tile([rows, 512], F32, tag="ey")
                for i in range(IT):
                    nc.tensor.matmul(
                        pe0[:],
                        g_t[:, i, t0 : t0 + rows],
                        fc2_t[:, i, 0:512],
                        start=(i == 0),
                        stop=(i == IT - 1),
                    )
                    nc.tensor.matmul(
                        pe1[:],
                        g_t[:, i, t0 : t0 + rows],
                        fc2_t[:, i, 512:1024],
                        start=(i == 0),
                        stop=(i == IT - 1),
                    )
                st0 = stpool.tile([rows, 512], BF16, tag="st")
                nc.vector.tensor_copy(st0[:], pe0[:])
                nc.sync.dma_start(yg_d[t0 : t0 + rows, 0:512], st0[:])
                st1 = stpool.tile([rows, 512], BF16, tag="st")
                nc.vector.tensor_copy(st1[:], pe1[:])
                nc.sync.dma_start(yg_d[t0 : t0 + rows, 512:1024], st1[:])

            # ---- shared down: ys[t, :] = sh_t[:, :, t].T @ dw ----
            for t in range(SSL // 128):
                t0 = t * 128
                pd0 = psey.tile([128, 512], F32, tag="ey")
                pd1 = psey.tile([128, 512], F32, tag="ey")
                for i in range(ST):
                    nc.tensor.matmul(
                        pd0[:],
                        sh_t[:, i, t0 : t0 + 128],
                        dw_t[:, i, 0:512],
                        start=(i == 0),
                        stop=(i == ST - 1),
                    )
                    nc.tensor.matmul(
                        pd1[:],
                        sh_t[:, i, t0 : t0 + 128],
                        dw_t[:, i, 512:1024],
                        start=(i == 0),
                        stop=(i == ST - 1),
                    )
                # quarter-width copies/DMAs shorten the post-matmul tail
                for q, ps in ((0, pd0), (1, pd0), (2, pd1), (3, pd1)):
                    c0 = (q % 2) * 256
                    stq = stpool.tile([128, 256], BF16, tag="st")
                    nc.vector.tensor_copy(stq[:], ps[:, c0 : c0 + 256])
                    nc.sync.dma_start(
                        ys_d[t0 : t0 + 128, q * 256 : (q + 1) * 256], stq[:]
                    )

    nc.compile()
    return nc


_CACHED = {}


def _route(x, w_router):
    """Host router: top-2 indices (ties -> lower index, like lax.top_k)
    and softmax weights over the top-2 logits."""
    logits = x.astype(np.float32) @ w_router.astype(np.float32)  # [N, E]
    top2 = np.argsort(-logits, axis=1, kind="stable")[:, :TOPK]  # [N, 2]
    l2 = np.take_along_axis(logits, top2, axis=1)
    m = l2.max(axis=1, keepdims=True)
    ex = np.exp(l2 - m)
    w = ex / ex.sum(axis=1, keepdims=True)
    return top2, w


def _km(a):
    """[H_like, C] -> [128, H_like//128, C] k-major contiguous."""
    kt = a.shape[0] // 128
    return np.ascontiguousarray(a.reshape(kt, 128, a.shape[1]).transpose(1, 0, 2))


def _prep(hidden_states, w_router, fc1_w, fc2_w, gate_w, up_w, down_w):
    import ml_dtypes

    bf16 = ml_dtypes.bfloat16
    x = np.ascontiguousarray(hidden_states.reshape(-1, H), dtype=np.float32)
    top2, w = _route(x, w_router)

    tok_lists = []
    wt_lists = []
    for e in range(NCORES):
        sel = np.where((top2[:, 0] == e) | (top2[:, 1] == e))[0]
        tok_lists.append(sel)
        wt_lists.append(np.where(top2[sel, 0] == e, w[sel, 0], w[sel, 1]))
    max_cnt = max(len(s) for s in tok_lists)
    cap = max(128, -(-max_cnt // 16) * 16)

    xb = x.astype(bf16)
    # gate/up 128-col pairs, shared across cores: [ST, 128, KT, 256]
    gu = np.empty((ST, 128, KT, 256), dtype=bf16)
    gwb = gate_w.astype(bf16)
    uwb = up_w.astype(bf16)
    for o in range(ST):
        gu[o, :, :, 0:128] = _km(gwb[:, o * 128 : (o + 1) * 128])
        gu[o, :, :, 128:256] = _km(uwb[:, o * 128 : (o + 1) * 128])
    dwk = _km(down_w.astype(bf16))  # [128, ST, H]

    in_maps = []
    for e in range(NCORES):
        sel = tok_lists[e]
        xgT = np.zeros((H, cap), dtype=bf16)
        xgT[:, : len(sel)] = xb[sel].T
        f1b = fc1_w[e].astype(bf16)
        f1 = np.empty((IT, 128, KT, 256), dtype=bf16)
        for j in range(IT):
            f1[j, :, :, 0:128] = _km(f1b[:, j * 128 : (j + 1) * 128])
            f1[j, :, :, 128:256] = _km(f1b[:, 1024 + j * 128 : 1024 + (j + 1) * 128])
        in_maps.append(
            {
                "xg": _km(xgT),
                "xs": _km(np.ascontiguousarray(xb[e * SSL : (e + 1) * SSL].T)),
                "fc1": f1,
                "fc2": _km(fc2_w[e].astype(bf16)),
                "gwu": gu,
                "dw": dwk,
            }
        )
    return cap, in_maps, tok_lists, wt_lists


def _assemble(results, tok_lists, wt_lists, orig_shape):
    out = np.zeros((N, H), dtype=np.float32)
    for e, res in enumerate(results):
        out[e * SSL : (e + 1) * SSL] = np.asarray(res["ys"]).astype(np.float32)
    for e, res in enumerate(results):
        sel = tok_lists[e]
        if len(sel) == 0:
            continue
        yg = np.asarray(res["yg"])[: len(sel)].astype(np.float32)
        out[sel] += wt_lists[e][:, None] * yg
    return out.reshape(orig_shape)


def kernel(hidden_states, w_router, fc1_w, fc2_w, gate_w, up_w, down_w):
    from concourse.bass_utils import run_bass_kernel_spmd

    cap, in_maps, tok_lists, wt_lists = _prep(
        hidden_states, w_router, fc1_w, fc2_w, gate_w, up_w, down_w
    )
    if cap not in _CACHED:
        _CACHED[cap] = build(cap)
    nc = _CACHED[cap]
    res = run_bass_kernel_spmd(nc, in_maps, core_ids=list(range(NCORES)))
    return _assemble(res.results, tok_lists, wt_lists, hidden_states.shape)
